# revision 9
# baseline (speedup 1.0000x reference)
"""Trainium2 Bass kernel for nn_AttentionNetwork (B=16, S=H=1024).

reference:
    energy  = tanh(concat([ht bcast, enc], -1) @ W_attn.T + b_attn)   [B,S,H]
    att     = softmax(energy, axis=1)  (over the seq axis)
    context = einsum('bsk,bkh->bsh', att, enc)
    returns (context, att)   (the W_v projection output is dead code)

Strategy (v5):
  - Data-parallel over batch: 2 batches per NeuronCore x 8 cores (SPMD).
  - htE = ht @ W1.T + b_attn computed on HOST (tiny GEMM), shipped as an
    8KB f32 tensor; device only does the two big GEMMs per batch.
  - mm1 computes energy TRANSPOSED (energyT[h,s]) so softmax over s is a
    free-dim reduction. kt-OUTER loop order over all 8 row-blocks (8 PSUM
    banks); encT is packed HALF-MAJOR on the host and the first chunks of
    the critical (b0, half0) stream are PER-KT so the very first matmul
    row only waits for ~384KB; w2tk (per-kt chunks) and encT stream on
    different DMA queues in parallel. PE warm-up dummies abut the first
    real matmul so HAM never re-throttles.
  - softmax: tanh(+bias) PSUM->SBUF f32 on ACT, exp with accum_out sums,
    reciprocal + at = ex*rec (bf16) on DVE (GpSimd tensor ops are slow
    and also wedge DVE's fast mode -- keep GpSimd to DMA triggers only).
  - att is written to DRAM TRANSPOSED (attT[h,s], bf16) straight from the
    at tiles -- no PE transposes; the host transposes + upcasts.
  - mm2: ctx = matmul(lhsT=at, rhs=enc) natural [s,h]; PSUM evacuated as
    bf16 via scalar/vector copy halves; DMA out bf16, host upcasts. The
    final group drains in 256-col chunks to shorten the tail.
  - Engine queues: scalar = activations/copies ONLY (DMA triggers would
    head-of-line block the tanh chain); sync = encT in + ctx out;
    gpsimd = htE/w2tk/enc in + attT out; vector = recip/at-mult/ctx-copy.
  - PE program order: warm dummies, mm1(b0), mm1(b1), mm2(b0), mm2(b1)
    with no PE gaps; everything else rides in the matmul shadow.
"""

import sys
import numpy as np

sys.path.insert(0, "/opt/trn_rl_repo")

import concourse.bass as bass
import concourse.mybir as mybir
import concourse.tile as tile
from concourse.bass_utils import run_bass_kernel_spmd
from concourse.masks import make_identity

F32 = mybir.dt.float32
BF = mybir.dt.bfloat16
AF = mybir.ActivationFunctionType

B, S, H = 16, 1024, 1024
NCORES = 8
BPC = B // NCORES  # batches per core
KT = 8             # 128-row contraction tiles
MT = 8             # output partition tiles
NH = 512           # matmul free-dim chunk (one PSUM bank fp32)
WARM_MM = 80       # [128,512] dummy matmuls bridging the DMA head


def _split_sync_waits(nc, maxw=1):
    """This walrus rejects instructions with more than one sync wait.
    Move excess on_wait entries onto InstNoOp on the same engine queue
    (executed in order ahead of the real instruction)."""
    ctr = 0
    for fn in nc.m.functions:
        for blk in fn.blocks:
            new = []
            for inst in blk.instructions:
                si = inst.sync_info
                if si is not None and si.on_wait and len(si.on_wait) > maxw:
                    waits = list(si.on_wait)
                    extra, keep = waits[:-maxw], waits[-maxw:]
                    for i in range(0, len(extra), maxw):
                        ctr += 1
                        nop = mybir.InstNoOp(
                            name=f"I-ws-{ctr}",
                            engine=inst.engine,
                            sync_info=mybir.SyncInfo(
                                on_wait=extra[i : i + maxw], on_update=[]
                            ),
                        )
                        nc.register_instruction(nop)
                        new.append(nop)
                    inst.sync_info = mybir.SyncInfo(
                        on_wait=keep, on_update=list(si.on_update)
                    )
                new.append(inst)
            blk.instructions[:] = new
    return ctr


def build():
    nc = bass.Bass()
    # encTh[b, half, p, kt*NH + j] = enc[b][half*NH + j, kt*128 + p]
    encTh_d = nc.declare_dram_parameter(
        "encTh", [BPC, 2, 128, KT * NH], BF, isOutput=False
    )
    enc_d = nc.declare_dram_parameter("enc", [BPC, 128, KT * H], BF, isOutput=False)
    w2tk_d = nc.declare_dram_parameter("w2tk", [128, KT * H], BF, isOutput=False)
    htE_d = nc.declare_dram_parameter("htE", [128, MT * BPC], F32, isOutput=False)
    ctx_d = nc.declare_dram_parameter("ctx", [BPC, S, H], BF, isOutput=True)
    attT_d = nc.declare_dram_parameter("attT", [BPC, H, S], BF, isOutput=True)

    with tile.TileContext(nc) as tc:
        with (
            tc.tile_pool(name="wpool", bufs=1) as wpool,
            tc.tile_pool(name="w2pool", bufs=KT) as w2pool,           # w2tk per-kt
            tc.tile_pool(name="etpool", bufs=16) as etpool,           # encT chunks
            tc.tile_pool(name="enpool", bufs=2 * KT // 4) as enpool,  # enc 4-kt
            tc.tile_pool(name="epool", bufs=MT) as epool,             # eT f32
            tc.tile_pool(name="xpool", bufs=3) as xpool,              # ex bf16
            tc.tile_pool(name="apool", bufs=2 * KT) as apool,         # at bf16
            tc.tile_pool(name="spool", bufs=4) as spool,              # sums/rec
            tc.tile_pool(name="cstg", bufs=4) as cstg,                # ctx staging
            tc.tile_pool(name="ps", bufs=8, space="PSUM") as ps,      # all 8 banks
        ):
            # --- ACT table warm first: nothing sits in front of it on the
            # scalar queue, so the 1.3us spline-table load happens at ~4us.
            warma = wpool.tile([128, 1], F32)
            nc.vector.memset(warma[:], 0.5)
            nc.scalar.activation(warma[:], warma[:], AF.Exp)

            # --- warmup prerequisites before any DMA trigger so the PE can
            # start its HAM-warming dummies as early as possible.
            dummy = wpool.tile([128, NH], BF)
            nc.vector.memset(dummy[:], 0.0)
            ident = wpool.tile([128, 128], BF)
            make_identity(nc, ident[:])

            # --- head DMAs, first-use order, two parallel bulk queues:
            # sync queue:   encTh(b0,h0) per-kt head chunks, then the rest of
            #               encTh; later ctx out.
            # gpsimd queue: htE (tiny), w2tk per-kt, enc b0/b1; attT out.
            # encT[b][half] -> list of (tile, kt_base, n_kt)
            encT = [[None, None] for _ in range(BPC)]

            def load_encT(b, half, kt_chunks):
                chunks = []
                kt0 = 0
                for n in kt_chunks:
                    et = etpool.tile(
                        [128, n * NH], BF, tag="encT", name=f"encT_{b}_{half}_{kt0}"
                    )
                    nc.sync.dma_start(
                        out=et[:],
                        in_=encTh_d[b, half, :, kt0 * NH : (kt0 + n) * NH],
                    )
                    chunks.append((et, kt0, n))
                    kt0 += n
                assert kt0 == KT
                encT[b][half] = chunks

            def encT_slice(b, half, kt):
                for et, kt0, n in encT[b][half]:
                    if kt0 <= kt < kt0 + n:
                        off = (kt - kt0) * NH
                        return et[:, off : off + NH]
                raise AssertionError

            w2tk = [None] * KT
            htE = wpool.tile([128, MT * BPC], F32)
            nc.gpsimd.dma_start(out=htE[:], in_=htE_d[:])
            # critical head: interleave issue of the two streams
            load_encT(0, 0, [1, 1, 2, 4])
            for kt in range(KT):
                wt = w2pool.tile([128, H], BF, tag="w2tk", name=f"w2tk_{kt}")
                nc.gpsimd.dma_start(
                    out=wt[:], in_=w2tk_d[:, kt * H : (kt + 1) * H]
                )
                w2tk[kt] = wt
            load_encT(0, 1, [4, 4])
            load_encT(1, 0, [4, 4])
            load_encT(1, 1, [4, 4])
            enc = [[None] * (KT // 4) for _ in range(BPC)]
            for b in range(BPC):
                for kq in range(KT // 4):
                    e = enpool.tile([128, 4 * H], BF, tag="enc", name=f"enc_{b}_{kq}")
                    nc.gpsimd.dma_start(
                        out=e[:], in_=enc_d[b, :, kq * 4 * H : (kq + 1) * 4 * H]
                    )
                    enc[b][kq] = e

            # --- PE warmup: keep HAM warm while the first chunks stream in.
            warmp = ps.tile([128, NH], F32, tag="ps")
            for i in range(WARM_MM):
                nc.tensor.matmul(
                    warmp[:], ident[:], dummy[:], start=(i == 0), stop=(i == WARM_MM - 1)
                )

            def mm1(b):
                # energyT[h,s] block-row mt: sum_kt w2tk[kt]^T @ encT[kt]
                pss = [None] * MT
                eTs = [None] * MT
                for half in range(2):
                    cs0 = half * NH
                    for mt in range(MT):
                        if half == 0:
                            eTs[mt] = epool.tile(
                                [128, S], F32, tag="eT", name=f"eT_{b}_{mt}"
                            )
                    for kt in range(KT):
                        wt = w2tk[kt]
                        et = encT_slice(b, half, kt)
                        for mt in range(MT):
                            if kt == 0:
                                pss[mt] = ps.tile(
                                    [128, NH], F32, tag="ps", name=f"ps_{b}_{half}_{mt}"
                                )
                            nc.tensor.matmul(
                                pss[mt][:],
                                wt[:, mt * 128 : (mt + 1) * 128],
                                et,
                                start=(kt == 0),
                                stop=(kt == KT - 1),
                            )
                            if kt == KT - 1:
                                # evacuate: eT = tanh(psum + htE[:,mt,b])
                                nc.scalar.activation(
                                    eTs[mt][:, cs0 : cs0 + NH],
                                    pss[mt][:],
                                    AF.Tanh,
                                    bias=htE[:, mt * BPC + b : mt * BPC + b + 1],
                                )
                return eTs

            def softmax(b, eTs):
                sums = spool.tile([128, MT], F32, tag="sums")
                rec = spool.tile([128, MT], F32, tag="rec")
                ats = []
                for mt in range(MT):
                    ex = xpool.tile([128, S], BF, tag="ex")
                    nc.scalar.activation(
                        ex[:], eTs[mt][:], AF.Exp, accum_out=sums[:, mt : mt + 1]
                    )
                    nc.vector.reciprocal(rec[:, mt : mt + 1], sums[:, mt : mt + 1])
                    at = apool.tile([128, S], BF, tag="at")
                    nc.vector.tensor_scalar_mul(at[:], ex[:], rec[:, mt : mt + 1])
                    nc.gpsimd.dma_start(
                        out=attT_d[b, mt * 128 : (mt + 1) * 128, :], in_=at[:]
                    )
                    ats.append(at)
                return ats

            def mm2(b, ats):
                for mt2 in range(MT):
                    p0 = ps.tile([128, NH], F32, tag="ps", name=f"p0_{b}_{mt2}")
                    p1 = ps.tile([128, NH], F32, tag="ps", name=f"p1_{b}_{mt2}")
                    for kt in range(KT):
                        lhs = ats[kt][:, mt2 * 128 : (mt2 + 1) * 128]
                        en = enc[b][kt // 4]
                        eo = (kt % 4) * H
                        nc.tensor.matmul(
                            p0[:], lhs, en[:, eo : eo + NH],
                            start=(kt == 0), stop=(kt == KT - 1),
                        )
                        nc.tensor.matmul(
                            p1[:], lhs, en[:, eo + NH : eo + H],
                            start=(kt == 0), stop=(kt == KT - 1),
                        )
                    stg = cstg.tile([128, H], BF, tag="cstg")
                    last = b == 1 and mt2 == MT - 1
                    if not last:
                        nc.scalar.copy(out=stg[:, :NH], in_=p0[:])
                        nc.sync.dma_start(
                            out=ctx_d[b, mt2 * 128 : (mt2 + 1) * 128, :NH],
                            in_=stg[:, :NH],
                        )
                        nc.vector.tensor_copy(stg[:, NH:], p1[:])
                        nc.sync.dma_start(
                            out=ctx_d[b, mt2 * 128 : (mt2 + 1) * 128, NH:],
                            in_=stg[:, NH:],
                        )
                    else:
                        # final group: drain in 256-col chunks, alternating
                        # scalar/vector, so the post-matmul tail is short
                        for q in range(4):
                            src = (p0 if q < 2 else p1)[:, (q % 2) * 256 : (q % 2) * 256 + 256]
                            dst = stg[:, q * 256 : (q + 1) * 256]
                            if q % 2 == 0:
                                nc.scalar.copy(out=dst, in_=src)
                            else:
                                nc.vector.tensor_copy(dst, src)
                            nc.sync.dma_start(
                                out=ctx_d[b, mt2 * 128 : (mt2 + 1) * 128,
                                          q * 256 : (q + 1) * 256],
                                in_=dst,
                            )

            eT0 = mm1(0)
            a0 = softmax(0, eT0)
            eT1 = mm1(1)
            mm2(0, a0)
            a1 = softmax(1, eT1)
            mm2(1, a1)

    _split_sync_waits(nc, 1)
    return nc


_NC_CACHE = {}


def _get_nc():
    if "nc" not in _NC_CACHE:
        _NC_CACHE["nc"] = build()
    return _NC_CACHE["nc"]


def _pack(m):
    # [1024, D] -> [128, 8*D] with 128-row tile kt at columns [kt*D,(kt+1)*D)
    d = m.shape[1]
    return np.ascontiguousarray(
        m.reshape(KT, 128, d).transpose(1, 0, 2).reshape(128, KT * d)
    )


def _make_in_maps(ht, enc, W_attn, b_attn):
    import ml_dtypes

    bf = ml_dtypes.bfloat16
    ht = np.asarray(ht, np.float32)
    enc = np.asarray(enc, np.float32)
    W = np.asarray(W_attn, np.float32)
    ba = np.asarray(b_attn, np.float32)

    # w2tk[p, kt*H + mt*128 + j] = W2T[kt*128+p, mt*128+j] (kt-major packing)
    w2tk_p = _pack(W[:, H:].T.copy()).astype(bf)
    # htE_full[b, h] = ht @ W1.T + b_attn  (computed on host, tiny)
    htE_full = (ht @ W[:, :H].T + ba).astype(np.float32)  # [B, H]

    in_maps = []
    for c in range(NCORES):
        bs = slice(BPC * c, BPC * (c + 1))
        enc_c = enc[bs]
        enc_p = np.stack([_pack(enc_c[i]) for i in range(BPC)]).astype(bf)
        # encTh[b, half, p, kt*NH + j] = enc_c[b][half*NH + j, kt*128 + p]
        encTh_p = np.ascontiguousarray(
            enc_c.reshape(BPC, 2, NH, KT, 128).transpose(0, 1, 4, 3, 2)
        ).reshape(BPC, 2, 128, KT * NH).astype(bf)
        # htE_col[p, mt*BPC + i] = htE_full[bs][i, mt*128 + p]
        htE_c = np.ascontiguousarray(
            htE_full[bs].reshape(BPC, MT, 128).transpose(2, 1, 0).reshape(128, MT * BPC)
        )
        in_maps.append(
            {"enc": enc_p, "encTh": encTh_p, "w2tk": w2tk_p, "htE": htE_c}
        )
    return in_maps


def _run(in_maps, trace=False):
    res = run_bass_kernel_spmd(
        _get_nc(), in_maps, core_ids=list(range(NCORES)), trace=trace
    )
    ctx = np.concatenate(
        [r["ctx"].astype(np.float32) for r in res.results], axis=0
    )
    att = np.concatenate(
        [r["attT"].transpose(0, 2, 1).astype(np.float32) for r in res.results],
        axis=0,
    )
    return (ctx, att), res


def kernel(ht, encoder_out, W_attn, b_attn, W_v=None, **_unused):
    out, _ = _run(_make_in_maps(ht, encoder_out, W_attn, b_attn), trace=False)
    return out


def kernel_traced(ht, encoder_out, W_attn, b_attn, W_v=None, **_unused):
    """Like kernel() but also returns the BassKernelResults with profile."""
    out, res = _run(_make_in_maps(ht, encoder_out, W_attn, b_attn), trace=True)
    return out, res


# revision 11
# speedup vs baseline: 1.0839x; 1.0839x over previous
"""Trainium2 Bass kernel for nn_AttentionNetwork (B=16, S=H=1024).

reference:
    energy  = tanh(concat([ht bcast, enc], -1) @ W_attn.T + b_attn)   [B,S,H]
    att     = softmax(energy, axis=1)  (over the seq axis)
    context = einsum('bsk,bkh->bsh', att, enc)
    returns (context, att)   (the W_v projection output is dead code)

Strategy (v5):
  - Data-parallel over batch: 2 batches per NeuronCore x 8 cores (SPMD).
  - htE = ht @ W1.T + b_attn computed on HOST (tiny GEMM), shipped as an
    8KB f32 tensor; device only does the two big GEMMs per batch.
  - mm1 computes energy TRANSPOSED (energyT[h,s]) so softmax over s is a
    free-dim reduction. kt-OUTER loop order over all 8 row-blocks (8 PSUM
    banks); encT is packed HALF-MAJOR on the host and the first chunks of
    the critical (b0, half0) stream are PER-KT so the very first matmul
    row only waits for ~384KB; w2tk (per-kt chunks) and encT stream on
    different DMA queues in parallel. PE warm-up dummies abut the first
    real matmul so HAM never re-throttles.
  - softmax: tanh(+bias) PSUM->SBUF f32 on ACT, exp with accum_out sums,
    reciprocal + at = ex*rec (bf16) on DVE (GpSimd tensor ops are slow
    and also wedge DVE's fast mode -- keep GpSimd to DMA triggers only).
  - att is written to DRAM TRANSPOSED (attT[h,s], bf16) straight from the
    at tiles -- no PE transposes; the host transposes + upcasts.
  - mm2: ctx = matmul(lhsT=at, rhs=enc) natural [s,h]; PSUM evacuated as
    bf16 via scalar/vector copy halves; DMA out bf16, host upcasts. The
    final group drains in 256-col chunks to shorten the tail.
  - Engine queues: scalar = activations/copies ONLY (DMA triggers would
    head-of-line block the tanh chain); sync = encT in + ctx out;
    gpsimd = htE/w2tk/enc in + attT out; vector = recip/at-mult/ctx-copy.
  - PE program order: warm dummies, mm1(b0), mm1(b1), mm2(b0), mm2(b1)
    with no PE gaps; everything else rides in the matmul shadow.
"""

import sys
import numpy as np

sys.path.insert(0, "/opt/trn_rl_repo")

import concourse.bass as bass
import concourse.mybir as mybir
import concourse.tile as tile
from concourse.bass_utils import run_bass_kernel_spmd
from concourse.masks import make_identity

F32 = mybir.dt.float32
BF = mybir.dt.bfloat16
AF = mybir.ActivationFunctionType

B, S, H = 16, 1024, 1024
NCORES = 8
BPC = B // NCORES  # batches per core
KT = 8             # 128-row contraction tiles
MT = 8             # output partition tiles
NH = 512           # matmul free-dim chunk (one PSUM bank fp32)
WARM_MM = 14       # [128,512] dummy matmuls bridging the DMA head


def _split_sync_waits(nc, maxw=1):
    """This walrus rejects instructions with more than one sync wait.
    Move excess on_wait entries onto InstNoOp on the same engine queue
    (executed in order ahead of the real instruction)."""
    ctr = 0
    for fn in nc.m.functions:
        for blk in fn.blocks:
            new = []
            for inst in blk.instructions:
                si = inst.sync_info
                if si is not None and si.on_wait and len(si.on_wait) > maxw:
                    waits = list(si.on_wait)
                    extra, keep = waits[:-maxw], waits[-maxw:]
                    for i in range(0, len(extra), maxw):
                        ctr += 1
                        nop = mybir.InstNoOp(
                            name=f"I-ws-{ctr}",
                            engine=inst.engine,
                            sync_info=mybir.SyncInfo(
                                on_wait=extra[i : i + maxw], on_update=[]
                            ),
                        )
                        nc.register_instruction(nop)
                        new.append(nop)
                    inst.sync_info = mybir.SyncInfo(
                        on_wait=keep, on_update=list(si.on_update)
                    )
                new.append(inst)
            blk.instructions[:] = new
    return ctr


def build():
    nc = bass.Bass()
    # encTh[b, half, p, kt*NH + j] = enc[b][half*NH + j, kt*128 + p]
    encTh_d = nc.declare_dram_parameter(
        "encTh", [BPC, 2, 128, KT * NH], BF, isOutput=False
    )
    enc_d = nc.declare_dram_parameter("enc", [BPC, 128, KT * H], BF, isOutput=False)
    w2tk_d = nc.declare_dram_parameter("w2tk", [128, KT * H], BF, isOutput=False)
    htE_d = nc.declare_dram_parameter("htE", [128, MT * BPC], F32, isOutput=False)
    ctx_d = nc.declare_dram_parameter("ctx", [BPC, S, H], BF, isOutput=True)
    attT_d = nc.declare_dram_parameter("attT", [BPC, H, S], BF, isOutput=True)

    with tile.TileContext(nc) as tc:
        with (
            tc.tile_pool(name="wpool", bufs=1) as wpool,
            tc.tile_pool(name="w2pool", bufs=KT) as w2pool,           # w2tk per-kt
            tc.tile_pool(name="etpool", bufs=16) as etpool,           # encT chunks
            tc.tile_pool(name="enpool", bufs=2 * KT // 4) as enpool,  # enc 4-kt
            tc.tile_pool(name="epool", bufs=MT) as epool,             # eT f32
            tc.tile_pool(name="xpool", bufs=3) as xpool,              # ex bf16
            tc.tile_pool(name="apool", bufs=2 * KT) as apool,         # at bf16
            tc.tile_pool(name="spool", bufs=4) as spool,              # sums/rec
            tc.tile_pool(name="cstg", bufs=4) as cstg,                # ctx staging
            tc.tile_pool(name="ps", bufs=8, space="PSUM") as ps,      # all 8 banks
        ):
            # --- ACT table warm first: nothing sits in front of it on the
            # scalar queue, so the 1.3us spline-table load happens at ~4us.
            warma = wpool.tile([128, 1], F32)
            nc.vector.memset(warma[:], 0.5)
            nc.scalar.activation(warma[:], warma[:], AF.Exp)

            # --- warmup prerequisites before any DMA trigger so the PE can
            # start its HAM-warming dummies as early as possible (both on the
            # vector queue, which is otherwise idle at the head).
            dummy = wpool.tile([128, NH], BF)
            nc.vector.memset(dummy[:], 0.0)
            ident = wpool.tile([128, 128], BF)
            nc.vector.memset(ident[:], 0.0)

            # --- head DMAs, first-use order, two parallel bulk queues:
            # sync queue:   encTh(b0,h0) per-kt head chunks, then the rest of
            #               encTh; later ctx out.
            # gpsimd queue: htE (tiny), w2tk per-kt, enc b0/b1; attT out.
            # encT[b][half] -> list of (tile, kt_base, n_kt)
            encT = [[None, None] for _ in range(BPC)]

            def load_encT(b, half, kt_chunks):
                chunks = []
                kt0 = 0
                for n in kt_chunks:
                    et = etpool.tile(
                        [128, n * NH], BF, tag="encT", name=f"encT_{b}_{half}_{kt0}"
                    )
                    nc.sync.dma_start(
                        out=et[:],
                        in_=encTh_d[b, half, :, kt0 * NH : (kt0 + n) * NH],
                    )
                    chunks.append((et, kt0, n))
                    kt0 += n
                assert kt0 == KT
                encT[b][half] = chunks

            def encT_slice(b, half, kt):
                for et, kt0, n in encT[b][half]:
                    if kt0 <= kt < kt0 + n:
                        off = (kt - kt0) * NH
                        return et[:, off : off + NH]
                raise AssertionError

            w2tk = [None] * KT
            htE = wpool.tile([128, MT * BPC], F32)
            nc.gpsimd.dma_start(out=htE[:], in_=htE_d[:])
            # critical head: interleave issue of the two streams
            load_encT(0, 0, [1, 1, 2, 4])
            for kt in range(KT):
                wt = w2pool.tile([128, H], BF, tag="w2tk", name=f"w2tk_{kt}")
                nc.gpsimd.dma_start(
                    out=wt[:], in_=w2tk_d[:, kt * H : (kt + 1) * H]
                )
                w2tk[kt] = wt
            load_encT(0, 1, [4, 4])
            load_encT(1, 0, [4, 4])
            load_encT(1, 1, [4, 4])
            enc = [[None] * (KT // 4) for _ in range(BPC)]
            for b in range(BPC):
                for kq in range(KT // 4):
                    e = enpool.tile([128, 4 * H], BF, tag="enc", name=f"enc_{b}_{kq}")
                    nc.gpsimd.dma_start(
                        out=e[:], in_=enc_d[b, :, kq * 4 * H : (kq + 1) * 4 * H]
                    )
                    enc[b][kq] = e

            # --- PE warmup: keep HAM warm while the first chunks stream in.
            warmp = ps.tile([128, NH], F32, tag="ps")
            for i in range(WARM_MM):
                nc.tensor.matmul(
                    warmp[:], ident[:], dummy[:], start=(i == 0), stop=(i == WARM_MM - 1)
                )

            def mm1(b):
                # energyT[h,s] block-row mt: sum_kt w2tk[kt]^T @ encT[kt]
                pss = [None] * MT
                eTs = [None] * MT
                for half in range(2):
                    cs0 = half * NH
                    for mt in range(MT):
                        if half == 0:
                            eTs[mt] = epool.tile(
                                [128, S], F32, tag="eT", name=f"eT_{b}_{mt}"
                            )
                    for kt in range(KT):
                        wt = w2tk[kt]
                        et = encT_slice(b, half, kt)
                        for mt in range(MT):
                            if kt == 0:
                                pss[mt] = ps.tile(
                                    [128, NH], F32, tag="ps", name=f"ps_{b}_{half}_{mt}"
                                )
                            nc.tensor.matmul(
                                pss[mt][:],
                                wt[:, mt * 128 : (mt + 1) * 128],
                                et,
                                start=(kt == 0),
                                stop=(kt == KT - 1),
                            )
                            if kt == KT - 1:
                                # evacuate: eT = tanh(psum + htE[:,mt,b])
                                nc.scalar.activation(
                                    eTs[mt][:, cs0 : cs0 + NH],
                                    pss[mt][:],
                                    AF.Tanh,
                                    bias=htE[:, mt * BPC + b : mt * BPC + b + 1],
                                )
                return eTs

            def softmax(b, eTs):
                sums = spool.tile([128, MT], F32, tag="sums")
                rec = spool.tile([128, MT], F32, tag="rec")
                ats = []
                for mt in range(MT):
                    ex = xpool.tile([128, S], BF, tag="ex")
                    nc.scalar.activation(
                        ex[:], eTs[mt][:], AF.Exp, accum_out=sums[:, mt : mt + 1]
                    )
                    nc.vector.reciprocal(rec[:, mt : mt + 1], sums[:, mt : mt + 1])
                    at = apool.tile([128, S], BF, tag="at")
                    nc.vector.tensor_scalar_mul(at[:], ex[:], rec[:, mt : mt + 1])
                    nc.gpsimd.dma_start(
                        out=attT_d[b, mt * 128 : (mt + 1) * 128, :], in_=at[:]
                    )
                    ats.append(at)
                return ats

            def mm2(b, ats):
                for mt2 in range(MT):
                    p0 = ps.tile([128, NH], F32, tag="ps", name=f"p0_{b}_{mt2}")
                    p1 = ps.tile([128, NH], F32, tag="ps", name=f"p1_{b}_{mt2}")
                    for kt in range(KT):
                        lhs = ats[kt][:, mt2 * 128 : (mt2 + 1) * 128]
                        en = enc[b][kt // 4]
                        eo = (kt % 4) * H
                        nc.tensor.matmul(
                            p0[:], lhs, en[:, eo : eo + NH],
                            start=(kt == 0), stop=(kt == KT - 1),
                        )
                        nc.tensor.matmul(
                            p1[:], lhs, en[:, eo + NH : eo + H],
                            start=(kt == 0), stop=(kt == KT - 1),
                        )
                    stg = cstg.tile([128, H], BF, tag="cstg")
                    last = b == 1 and mt2 == MT - 1
                    if not last:
                        nc.scalar.copy(out=stg[:, :NH], in_=p0[:])
                        nc.sync.dma_start(
                            out=ctx_d[b, mt2 * 128 : (mt2 + 1) * 128, :NH],
                            in_=stg[:, :NH],
                        )
                        nc.vector.tensor_copy(stg[:, NH:], p1[:])
                        nc.sync.dma_start(
                            out=ctx_d[b, mt2 * 128 : (mt2 + 1) * 128, NH:],
                            in_=stg[:, NH:],
                        )
                    else:
                        # final group: drain in 256-col chunks, alternating
                        # scalar/vector, so the post-matmul tail is short
                        for q in range(4):
                            src = (p0 if q < 2 else p1)[:, (q % 2) * 256 : (q % 2) * 256 + 256]
                            dst = stg[:, q * 256 : (q + 1) * 256]
                            if q % 2 == 0:
                                nc.scalar.copy(out=dst, in_=src)
                            else:
                                nc.vector.tensor_copy(dst, src)
                            nc.sync.dma_start(
                                out=ctx_d[b, mt2 * 128 : (mt2 + 1) * 128,
                                          q * 256 : (q + 1) * 256],
                                in_=dst,
                            )

            eT0 = mm1(0)
            a0 = softmax(0, eT0)
            eT1 = mm1(1)
            mm2(0, a0)
            a1 = softmax(1, eT1)
            mm2(1, a1)

    _split_sync_waits(nc, 1)
    return nc


_NC_CACHE = {}


def _get_nc():
    if "nc" not in _NC_CACHE:
        _NC_CACHE["nc"] = build()
    return _NC_CACHE["nc"]


def _pack(m):
    # [1024, D] -> [128, 8*D] with 128-row tile kt at columns [kt*D,(kt+1)*D)
    d = m.shape[1]
    return np.ascontiguousarray(
        m.reshape(KT, 128, d).transpose(1, 0, 2).reshape(128, KT * d)
    )


def _make_in_maps(ht, enc, W_attn, b_attn):
    import ml_dtypes

    bf = ml_dtypes.bfloat16
    ht = np.asarray(ht, np.float32)
    enc = np.asarray(enc, np.float32)
    W = np.asarray(W_attn, np.float32)
    ba = np.asarray(b_attn, np.float32)

    # w2tk[p, kt*H + mt*128 + j] = W2T[kt*128+p, mt*128+j] (kt-major packing)
    w2tk_p = _pack(W[:, H:].T.copy()).astype(bf)
    # htE_full[b, h] = ht @ W1.T + b_attn  (computed on host, tiny)
    htE_full = (ht @ W[:, :H].T + ba).astype(np.float32)  # [B, H]

    in_maps = []
    for c in range(NCORES):
        bs = slice(BPC * c, BPC * (c + 1))
        enc_c = enc[bs]
        enc_p = np.stack([_pack(enc_c[i]) for i in range(BPC)]).astype(bf)
        # encTh[b, half, p, kt*NH + j] = enc_c[b][half*NH + j, kt*128 + p]
        encTh_p = np.ascontiguousarray(
            enc_c.reshape(BPC, 2, NH, KT, 128).transpose(0, 1, 4, 3, 2)
        ).reshape(BPC, 2, 128, KT * NH).astype(bf)
        # htE_col[p, mt*BPC + i] = htE_full[bs][i, mt*128 + p]
        htE_c = np.ascontiguousarray(
            htE_full[bs].reshape(BPC, MT, 128).transpose(2, 1, 0).reshape(128, MT * BPC)
        )
        in_maps.append(
            {"enc": enc_p, "encTh": encTh_p, "w2tk": w2tk_p, "htE": htE_c}
        )
    return in_maps


def _run(in_maps, trace=False):
    res = run_bass_kernel_spmd(
        _get_nc(), in_maps, core_ids=list(range(NCORES)), trace=trace
    )
    ctx = np.concatenate(
        [r["ctx"].astype(np.float32) for r in res.results], axis=0
    )
    att = np.concatenate(
        [r["attT"].transpose(0, 2, 1).astype(np.float32) for r in res.results],
        axis=0,
    )
    return (ctx, att), res


def kernel(ht, encoder_out, W_attn, b_attn, W_v=None, **_unused):
    out, _ = _run(_make_in_maps(ht, encoder_out, W_attn, b_attn), trace=False)
    return out


def kernel_traced(ht, encoder_out, W_attn, b_attn, W_v=None, **_unused):
    """Like kernel() but also returns the BassKernelResults with profile."""
    out, res = _run(_make_in_maps(ht, encoder_out, W_attn, b_attn), trace=True)
    return out, res


# revision 54
# speedup vs baseline: 1.1156x; 1.0292x over previous
"""Trainium2 Bass kernel for nn_AttentionNetwork (B=16, S=H=1024).

reference:
    energy  = tanh(concat([ht bcast, enc], -1) @ W_attn.T + b_attn)   [B,S,H]
    att     = softmax(energy, axis=1)  (over the seq axis)
    context = einsum('bsk,bkh->bsh', att, enc)
    returns (context, att)   (the W_v projection output is dead code)

Strategy (final):
  - Data-parallel over batch: 2 batches per NeuronCore x 8 cores (SPMD).
  - htE = ht @ W1.T + b_attn computed on HOST (tiny GEMM), shipped as an
    8KB f32 tensor; device only does the two big GEMMs per batch.
  - mm1 computes energy TRANSPOSED (energyT[h,s]) so softmax over s is a
    free-dim reduction. kt-OUTER loop order over all 8 row-blocks (8 PSUM
    banks): each arriving chunk unlocks a whole row of work, so pass 1 is
    perfectly DMA-paced with no PE gaps. encT is packed HALF-MAJOR on the
    host and the critical (b0, half0) stream uses PER-KT 128KB chunks.
  - Input streams ride two DMA queues in parallel (sync: encT; gpsimd:
    htE/w2tk/enc-prefetch); the sync queue is empirically much faster, so
    it carries the tightest deadlines. Starting the PE earlier than the
    streams can sustain only creates gaps + HAM re-throttle; instead PE
    warm-up dummies (8 big + 34 small) bridge exactly until first data.
  - softmax: tanh(+bias) PSUM->SBUF f32 on ACT, exp with accum_out sums,
    reciprocal + at = ex*rec (bf16) on DVE (GpSimd tensor ops are slow
    and also wedge DVE's fast mode -- keep GpSimd to DMA triggers only).
  - att is written to DRAM TRANSPOSED (attT[h,s], bf16) straight from the
    at tiles -- no PE transposes; the host transposes + upcasts.
  - mm2: ctx = matmul(lhsT=at, rhs=enc) natural [s,h]; PSUM evacuated as
    bf16 via scalar/vector copy halves; DMA out bf16 (halving write
    traffic), host upcasts. The final row-block runs its two column
    halves sequentially and drains into contiguous side tensors so the
    last DMA is a short 1D burst overlapped with the last matmuls.
  - Engine queues: scalar = activations/copies ONLY (DMA triggers would
    head-of-line block the tanh chain); sync = encT in + ctx out;
    gpsimd = htE/w2tk/enc in + attT out; vector = recip/at-mult/ctx-copy.
  - PE program order: warm dummies, mm1(b0), mm1(b1), mm2(b0), mm2(b1)
    with no PE gaps; everything else rides in the matmul shadow.
  (fp8/DoubleRow was evaluated and rejected: only e4m3/e5m2 double-pump,
  and their quantization noise puts ctx relL2 at ~4e-2 vs the 2e-2 gate.)
"""

import sys
import numpy as np

sys.path.insert(0, "/opt/trn_rl_repo")

import concourse.bass as bass
import concourse.mybir as mybir
import concourse.tile as tile
from concourse.bass_utils import run_bass_kernel_spmd

F32 = mybir.dt.float32
BF = mybir.dt.bfloat16
AF = mybir.ActivationFunctionType

B, S, H = 16, 1024, 1024
NCORES = 8
BPC = B // NCORES  # batches per core
KT = 8             # 128-row contraction tiles
MT = 8             # output partition tiles
NH = 512           # matmul free-dim chunk (one PSUM bank fp32)
WARM_BIG = 8       # [128,512] dummy matmuls warming the PE clock
WARM_SMALL = 34    # [128,128] dummy matmuls bridging until first data


def _split_sync_waits(nc, maxw=1):
    """This walrus rejects instructions with more than one sync wait.
    Move excess on_wait entries onto InstNoOp on the same engine queue
    (executed in order ahead of the real instruction)."""
    ctr = 0
    for fn in nc.m.functions:
        for blk in fn.blocks:
            new = []
            for inst in blk.instructions:
                si = inst.sync_info
                if si is not None and si.on_wait and len(si.on_wait) > maxw:
                    waits = list(si.on_wait)
                    extra, keep = waits[:-maxw], waits[-maxw:]
                    for i in range(0, len(extra), maxw):
                        ctr += 1
                        nop = mybir.InstNoOp(
                            name=f"I-ws-{ctr}",
                            engine=inst.engine,
                            sync_info=mybir.SyncInfo(
                                on_wait=extra[i : i + maxw], on_update=[]
                            ),
                        )
                        nc.register_instruction(nop)
                        new.append(nop)
                    inst.sync_info = mybir.SyncInfo(
                        on_wait=keep, on_update=list(si.on_update)
                    )
                new.append(inst)
            blk.instructions[:] = new
    return ctr


def build():
    nc = bass.Bass()
    # encTh[b, half, p, kt*NH + j] = enc[b][half*NH + j, kt*128 + p]
    encTh_d = nc.declare_dram_parameter(
        "encTh", [BPC, 2, 128, KT * NH], BF, isOutput=False
    )
    enc_d = nc.declare_dram_parameter("enc", [BPC, 128, KT * H], BF, isOutput=False)
    w2tk_d = nc.declare_dram_parameter("w2tk", [128, KT * H], BF, isOutput=False)
    htE_d = nc.declare_dram_parameter("htE", [128, MT * BPC], F32, isOutput=False)
    ctx_d = nc.declare_dram_parameter("ctx", [BPC, S, H], BF, isOutput=True)
    # the final row-block's two column-halves as contiguous tensors: the very
    # last DMA is a fast 1D 128KB burst instead of a strided 2D write
    ctxt_d = nc.declare_dram_parameter("ctxt", [2, 128, NH], BF, isOutput=True)
    attT_d = nc.declare_dram_parameter("attT", [BPC, H, S], BF, isOutput=True)

    with tile.TileContext(nc) as tc:
        with (
            tc.tile_pool(name="wpool", bufs=1) as wpool,
            tc.tile_pool(name="w2pool", bufs=KT) as w2pool,           # w2tk per-kt
            tc.tile_pool(name="etpool", bufs=16) as etpool,           # encT chunks
            tc.tile_pool(name="enpool", bufs=2 * KT // 4) as enpool,  # enc 4-kt
            tc.tile_pool(name="epool", bufs=MT) as epool,             # eT f32
            tc.tile_pool(name="xpool", bufs=3) as xpool,              # ex bf16
            tc.tile_pool(name="apool", bufs=2 * KT) as apool,         # at bf16
            tc.tile_pool(name="spool", bufs=4) as spool,              # sums/rec
            tc.tile_pool(name="cstg", bufs=4) as cstg,                # ctx staging
            tc.tile_pool(name="ps", bufs=8, space="PSUM") as ps,      # all 8 banks
        ):
            # --- ACT table warm first: nothing sits in front of it on the
            # scalar queue, so the 1.3us spline-table load happens at ~4us.
            warma = wpool.tile([128, 1], F32)
            nc.vector.memset(warma[:], 0.5)
            nc.scalar.activation(warma[:], warma[:], AF.Exp)

            # --- warmup prerequisites before any DMA trigger so the PE can
            # start its HAM-warming dummies as early as possible (both on the
            # vector queue, which is otherwise idle at the head).
            dummy = wpool.tile([128, NH], BF)
            nc.vector.memset(dummy[:], 0.0)
            ident = wpool.tile([128, 128], BF)
            nc.vector.memset(ident[:], 0.0)

            # --- head DMAs, first-use order, two parallel bulk queues:
            # sync queue:   encTh(b0,h0) per-kt head chunks, then the rest of
            #               encTh; later ctx out.
            # gpsimd queue: htE (tiny), w2tk per-kt, enc b0/b1; attT out.
            # encT[b][half] -> list of (tile, kt_base, n_kt)
            encT = [[None, None] for _ in range(BPC)]

            def load_encT(b, half, kt_chunks):
                chunks = []
                kt0 = 0
                for n in kt_chunks:
                    et = etpool.tile(
                        [128, n * NH], BF, tag="encT", name=f"encT_{b}_{half}_{kt0}"
                    )
                    nc.sync.dma_start(
                        out=et[:],
                        in_=encTh_d[b, half, :, kt0 * NH : (kt0 + n) * NH],
                    )
                    chunks.append((et, kt0, n))
                    kt0 += n
                assert kt0 <= KT
                encT[b][half] = chunks

            def encT_slice(b, half, kt):
                for et, kt0, n in encT[b][half]:
                    if kt0 <= kt < kt0 + n:
                        off = (kt - kt0) * NH
                        return et[:, off : off + NH]
                raise AssertionError

            # critical head: encTh(b0,h0) per-kt chunks on the (fast) sync
            # queue; w2tk per-kt on gpsimd; prefetch follows on both. Starting
            # pass 1 earlier than the input streams can sustain only creates
            # PE gaps (and HAM re-throttle) -- this split paces perfectly.
            htE = wpool.tile([128, MT * BPC], F32)
            nc.gpsimd.dma_start(out=htE[:], in_=htE_d[:])
            load_encT(0, 0, [1] * KT)
            w2tk = [None] * KT
            for kt in range(KT):
                wt = w2pool.tile([128, H], BF, tag="w2tk", name=f"w2tk_{kt}")
                nc.gpsimd.dma_start(
                    out=wt[:], in_=w2tk_d[:, kt * H : (kt + 1) * H]
                )
                w2tk[kt] = wt
            load_encT(0, 1, [2, 2, 2, 2])
            load_encT(1, 0, [4, 4])
            load_encT(1, 1, [4, 4])
            enc = [[None] * (KT // 4) for _ in range(BPC)]
            for b in range(BPC):
                for kq in range(KT // 4):
                    e = enpool.tile(
                        [128, 4 * H], BF, tag="enc", name=f"enc_{b}_{kq}"
                    )
                    nc.gpsimd.dma_start(
                        out=e[:], in_=enc_d[b, :, kq * 4 * H : (kq + 1) * 4 * H]
                    )
                    enc[b][kq] = e

            def w2slice(kt, mt):
                return w2tk[kt][:, mt * 128 : (mt + 1) * 128]

            # --- PE warmup: keep HAM warm while the first chunks stream in.
            # A few big matmuls warm the clock, then small 128-col ones give
            # fine-grained bridging so the handoff to real work is tight.
            warmp = ps.tile([128, NH], F32, tag="ps")
            for i in range(WARM_BIG):
                nc.tensor.matmul(
                    warmp[:], ident[:], dummy[:], start=(i == 0), stop=(i == WARM_BIG - 1)
                )
            for i in range(WARM_SMALL):
                nc.tensor.matmul(
                    warmp[:, :128], ident[:], dummy[:, :128],
                    start=(i == 0), stop=(i == WARM_SMALL - 1),
                )

            def mm1(b):
                # energyT[h,s] block-row mt: sum_kt w2tk[kt]^T @ encT[kt]
                pss = [None] * MT
                eTs = [None] * MT
                for half in range(2):
                    cs0 = half * NH
                    for mt in range(MT):
                        if half == 0:
                            eTs[mt] = epool.tile(
                                [128, S], F32, tag="eT", name=f"eT_{b}_{mt}"
                            )
                    for kt in range(KT):
                        et = encT_slice(b, half, kt)
                        for mt in range(MT):
                            if kt == 0:
                                pss[mt] = ps.tile(
                                    [128, NH], F32, tag="ps", name=f"ps_{b}_{half}_{mt}"
                                )
                            nc.tensor.matmul(
                                pss[mt][:],
                                w2slice(kt, mt),
                                et,
                                start=(kt == 0),
                                stop=(kt == KT - 1),
                            )
                            if kt == KT - 1:
                                # evacuate: eT = tanh(psum + htE[:,mt,b])
                                nc.scalar.activation(
                                    eTs[mt][:, cs0 : cs0 + NH],
                                    pss[mt][:],
                                    AF.Tanh,
                                    bias=htE[:, mt * BPC + b : mt * BPC + b + 1],
                                )
                return eTs

            def softmax(b, eTs):
                sums = spool.tile([128, MT], F32, tag="sums")
                rec = spool.tile([128, MT], F32, tag="rec")
                ats = []
                for mt in range(MT):
                    ex = xpool.tile([128, S], BF, tag="ex")
                    nc.scalar.activation(
                        ex[:], eTs[mt][:], AF.Exp, accum_out=sums[:, mt : mt + 1]
                    )
                    nc.vector.reciprocal(rec[:, mt : mt + 1], sums[:, mt : mt + 1])
                    at = apool.tile([128, S], BF, tag="at")
                    nc.vector.tensor_scalar_mul(at[:], ex[:], rec[:, mt : mt + 1])
                    nc.gpsimd.dma_start(
                        out=attT_d[b, mt * 128 : (mt + 1) * 128, :], in_=at[:]
                    )
                    ats.append(at)
                return ats

            def mm2(b, ats):
                for mt2 in range(MT):
                    p0 = ps.tile([128, NH], F32, tag="ps", name=f"p0_{b}_{mt2}")
                    p1 = ps.tile([128, NH], F32, tag="ps", name=f"p1_{b}_{mt2}")
                    last = b == 1 and mt2 == MT - 1
                    if last:
                        # sequence p0's 8 matmuls before p1's: p0 evacuates
                        # and DMAs while p1 still multiplies
                        for kt in range(KT):
                            nc.tensor.matmul(
                                p0[:],
                                ats[kt][:, mt2 * 128 : (mt2 + 1) * 128],
                                enc[b][kt // 4][:, (kt % 4) * H : (kt % 4) * H + NH],
                                start=(kt == 0), stop=(kt == KT - 1),
                            )
                        s0 = cstg.tile([128, NH], BF, tag="cstg")
                        nc.scalar.copy(out=s0[:], in_=p0[:])
                        nc.sync.dma_start(out=ctxt_d[0], in_=s0[:])
                        for kt in range(KT):
                            nc.tensor.matmul(
                                p1[:],
                                ats[kt][:, mt2 * 128 : (mt2 + 1) * 128],
                                enc[b][kt // 4][:, (kt % 4) * H + NH : (kt % 4) * H + H],
                                start=(kt == 0), stop=(kt == KT - 1),
                            )
                        s1 = cstg.tile([128, NH], BF, tag="cstg")
                        nc.vector.tensor_copy(s1[:], p1[:])
                        nc.sync.dma_start(out=ctxt_d[1], in_=s1[:])
                        continue
                    for kt in range(KT):
                        lhs = ats[kt][:, mt2 * 128 : (mt2 + 1) * 128]
                        en = enc[b][kt // 4]
                        eo = (kt % 4) * H
                        nc.tensor.matmul(
                            p0[:], lhs, en[:, eo : eo + NH],
                            start=(kt == 0), stop=(kt == KT - 1),
                        )
                        nc.tensor.matmul(
                            p1[:], lhs, en[:, eo + NH : eo + H],
                            start=(kt == 0), stop=(kt == KT - 1),
                        )
                    # separate half tiles so each DMA depends only on its
                    # own copy
                    s0 = cstg.tile([128, NH], BF, tag="cstg")
                    s1 = cstg.tile([128, NH], BF, tag="cstg")
                    nc.scalar.copy(out=s0[:], in_=p0[:])
                    nc.sync.dma_start(
                        out=ctx_d[b, mt2 * 128 : (mt2 + 1) * 128, :NH],
                        in_=s0[:],
                    )
                    nc.vector.tensor_copy(s1[:], p1[:])
                    nc.gpsimd.dma_start(
                        out=ctx_d[b, mt2 * 128 : (mt2 + 1) * 128, NH:],
                        in_=s1[:],
                    )

            eT0 = mm1(0)
            a0 = softmax(0, eT0)
            eT1 = mm1(1)
            mm2(0, a0)
            a1 = softmax(1, eT1)
            mm2(1, a1)

    _split_sync_waits(nc, 1)
    return nc


_NC_CACHE = {}


def _get_nc():
    if "nc" not in _NC_CACHE:
        _NC_CACHE["nc"] = build()
    return _NC_CACHE["nc"]


def _pack(m):
    # [1024, D] -> [128, 8*D] with 128-row tile kt at columns [kt*D,(kt+1)*D)
    d = m.shape[1]
    return np.ascontiguousarray(
        m.reshape(KT, 128, d).transpose(1, 0, 2).reshape(128, KT * d)
    )


def _make_in_maps(ht, enc, W_attn, b_attn):
    import ml_dtypes

    bf = ml_dtypes.bfloat16
    ht = np.asarray(ht, np.float32)
    enc = np.asarray(enc, np.float32)
    W = np.asarray(W_attn, np.float32)
    ba = np.asarray(b_attn, np.float32)

    # w2tk[p, kt*H + mt*128 + j] = W2T[kt*128+p, mt*128+j] (kt-major packing)
    w2tk_p = _pack(W[:, H:].T.copy()).astype(bf)
    # htE_full[b, h] = ht @ W1.T + b_attn  (computed on host, tiny)
    htE_full = (ht @ W[:, :H].T + ba).astype(np.float32)  # [B, H]

    in_maps = []
    for c in range(NCORES):
        bs = slice(BPC * c, BPC * (c + 1))
        enc_c = enc[bs]
        enc_p = np.stack([_pack(enc_c[i]) for i in range(BPC)]).astype(bf)
        # encTh[b, half, p, kt*NH + j] = enc_c[b][half*NH + j, kt*128 + p]
        encTh_p = np.ascontiguousarray(
            enc_c.reshape(BPC, 2, NH, KT, 128).transpose(0, 1, 4, 3, 2)
        ).reshape(BPC, 2, 128, KT * NH).astype(bf)
        # htE_col[p, mt*BPC + i] = htE_full[bs][i, mt*128 + p]
        htE_c = np.ascontiguousarray(
            htE_full[bs].reshape(BPC, MT, 128).transpose(2, 1, 0).reshape(128, MT * BPC)
        )
        in_maps.append(
            {"enc": enc_p, "encTh": encTh_p, "w2tk": w2tk_p, "htE": htE_c}
        )
    return in_maps


def _run(in_maps, trace=False):
    res = run_bass_kernel_spmd(
        _get_nc(), in_maps, core_ids=list(range(NCORES)), trace=trace
    )
    ctx_parts = []
    for r in res.results:
        c = r["ctx"].copy()
        tail = r["ctxt"]  # [2, 128, NH]: last row-block of b1, col-halves
        c[BPC - 1, (MT - 1) * 128 :, :NH] = tail[0]
        c[BPC - 1, (MT - 1) * 128 :, NH:] = tail[1]
        ctx_parts.append(c.astype(np.float32))
    ctx = np.concatenate(ctx_parts, axis=0)
    att = np.concatenate(
        [r["attT"].transpose(0, 2, 1).astype(np.float32) for r in res.results],
        axis=0,
    )
    return (ctx, att), res


def kernel(ht, encoder_out, W_attn, b_attn, W_v=None, **_unused):
    out, _ = _run(_make_in_maps(ht, encoder_out, W_attn, b_attn), trace=False)
    return out


def kernel_traced(ht, encoder_out, W_attn, b_attn, W_v=None, **_unused):
    """Like kernel() but also returns the BassKernelResults with profile."""
    out, res = _run(_make_in_maps(ht, encoder_out, W_attn, b_attn), trace=True)
    return out, res


# revision 55
# speedup vs baseline: 1.1158x; 1.0002x over previous
"""Trainium2 Bass kernel for nn_AttentionNetwork (B=16, S=H=1024).

reference:
    energy  = tanh(concat([ht bcast, enc], -1) @ W_attn.T + b_attn)   [B,S,H]
    att     = softmax(energy, axis=1)  (over the seq axis)
    context = einsum('bsk,bkh->bsh', att, enc)
    returns (context, att)   (the W_v projection output is dead code)

Strategy (final):
  - Data-parallel over batch: 2 batches per NeuronCore x 8 cores (SPMD).
  - htE = ht @ W1.T + b_attn computed on HOST (tiny GEMM), shipped as an
    8KB f32 tensor; device only does the two big GEMMs per batch.
  - mm1 computes energy TRANSPOSED (energyT[h,s]) so softmax over s is a
    free-dim reduction. kt-OUTER loop order over all 8 row-blocks (8 PSUM
    banks): each arriving chunk unlocks a whole row of work, so pass 1 is
    perfectly DMA-paced with no PE gaps. encT is packed HALF-MAJOR on the
    host and the critical (b0, half0) stream uses PER-KT 128KB chunks.
  - Input streams ride two DMA queues in parallel (sync: encT; gpsimd:
    htE/w2tk/enc-prefetch); the sync queue is empirically much faster, so
    it carries the tightest deadlines. Starting the PE earlier than the
    streams can sustain only creates gaps + HAM re-throttle; instead PE
    warm-up dummies (8 big + 34 small) bridge exactly until first data.
  - softmax: tanh(+bias) PSUM->SBUF f32 on ACT, exp with accum_out sums,
    reciprocal + at = ex*rec (bf16) on DVE (GpSimd tensor ops are slow
    and also wedge DVE's fast mode -- keep GpSimd to DMA triggers only).
  - att is written to DRAM TRANSPOSED (attT[h,s], bf16) straight from the
    at tiles -- no PE transposes; the host transposes + upcasts.
  - mm2: ctx = matmul(lhsT=at, rhs=enc) natural [s,h]; PSUM evacuated as
    bf16 via scalar/vector copy halves; DMA out bf16 (halving write
    traffic), host upcasts. The final row-block runs its two column
    halves sequentially and drains into contiguous side tensors so the
    last DMA is a short 1D burst overlapped with the last matmuls.
  - Engine queues: scalar = activations/copies ONLY (DMA triggers would
    head-of-line block the tanh chain); sync = encT in + ctx out;
    gpsimd = htE/w2tk/enc in + attT out; vector = recip/at-mult/ctx-copy.
  - PE program order: warm dummies, mm1(b0), mm1(b1), mm2(b0), mm2(b1)
    with no PE gaps; everything else rides in the matmul shadow.
  (fp8/DoubleRow was evaluated and rejected: only e4m3/e5m2 double-pump,
  and their quantization noise puts ctx relL2 at ~4e-2 vs the 2e-2 gate.)
"""

import sys
import numpy as np

sys.path.insert(0, "/opt/trn_rl_repo")

import concourse.bass as bass
import concourse.mybir as mybir
import concourse.tile as tile
from concourse.bass_utils import run_bass_kernel_spmd

F32 = mybir.dt.float32
BF = mybir.dt.bfloat16
AF = mybir.ActivationFunctionType

B, S, H = 16, 1024, 1024
NCORES = 8
BPC = B // NCORES  # batches per core
KT = 8             # 128-row contraction tiles
MT = 8             # output partition tiles
NH = 512           # matmul free-dim chunk (one PSUM bank fp32)
WARM_BIG = 8       # [128,512] dummy matmuls warming the PE clock
WARM_SMALL = 34    # [128,128] dummy matmuls bridging until first data


def _split_sync_waits(nc, maxw=1):
    """This walrus rejects instructions with more than one sync wait.
    Move excess on_wait entries onto InstNoOp on the same engine queue
    (executed in order ahead of the real instruction)."""
    ctr = 0
    for fn in nc.m.functions:
        for blk in fn.blocks:
            new = []
            for inst in blk.instructions:
                si = inst.sync_info
                if si is not None and si.on_wait and len(si.on_wait) > maxw:
                    waits = list(si.on_wait)
                    extra, keep = waits[:-maxw], waits[-maxw:]
                    for i in range(0, len(extra), maxw):
                        ctr += 1
                        nop = mybir.InstNoOp(
                            name=f"I-ws-{ctr}",
                            engine=inst.engine,
                            sync_info=mybir.SyncInfo(
                                on_wait=extra[i : i + maxw], on_update=[]
                            ),
                        )
                        nc.register_instruction(nop)
                        new.append(nop)
                    inst.sync_info = mybir.SyncInfo(
                        on_wait=keep, on_update=list(si.on_update)
                    )
                new.append(inst)
            blk.instructions[:] = new
    return ctr


def build():
    nc = bass.Bass()
    # encTh[b, half, p, kt*NH + j] = enc[b][half*NH + j, kt*128 + p]
    encTh_d = nc.declare_dram_parameter(
        "encTh", [BPC, 2, 128, KT * NH], BF, isOutput=False
    )
    enc_d = nc.declare_dram_parameter("enc", [BPC, 128, KT * H], BF, isOutput=False)
    w2tk_d = nc.declare_dram_parameter("w2tk", [128, KT * H], BF, isOutput=False)
    htE_d = nc.declare_dram_parameter("htE", [128, MT * BPC], F32, isOutput=False)
    ctx_d = nc.declare_dram_parameter("ctx", [BPC, S, H], BF, isOutput=True)
    # the final row-block's two column-halves as contiguous tensors: the very
    # last DMA is a fast 1D 128KB burst instead of a strided 2D write
    ctxt_d = nc.declare_dram_parameter("ctxt", [2, 128, NH], BF, isOutput=True)
    attT_d = nc.declare_dram_parameter("attT", [BPC, H, S], BF, isOutput=True)

    with tile.TileContext(nc) as tc:
        with (
            tc.tile_pool(name="wpool", bufs=1) as wpool,
            tc.tile_pool(name="w2pool", bufs=KT) as w2pool,           # w2tk per-kt
            tc.tile_pool(name="etpool", bufs=16) as etpool,           # encT chunks
            tc.tile_pool(name="enpool", bufs=2 * KT // 4) as enpool,  # enc 4-kt
            tc.tile_pool(name="epool", bufs=MT) as epool,             # eT f32
            tc.tile_pool(name="xpool", bufs=3) as xpool,              # ex bf16
            tc.tile_pool(name="apool", bufs=2 * KT) as apool,         # at bf16
            tc.tile_pool(name="spool", bufs=4) as spool,              # sums/rec
            tc.tile_pool(name="cstg", bufs=4) as cstg,                # ctx staging
            tc.tile_pool(name="ps", bufs=8, space="PSUM") as ps,      # all 8 banks
        ):
            # --- ACT table warm first: nothing sits in front of it on the
            # scalar queue, so the 1.3us spline-table load happens at ~4us.
            warma = wpool.tile([128, 1], F32)
            nc.vector.memset(warma[:], 0.5)
            nc.scalar.activation(warma[:], warma[:], AF.Exp)

            # --- warmup prerequisites before any DMA trigger so the PE can
            # start its HAM-warming dummies as early as possible (both on the
            # vector queue, which is otherwise idle at the head).
            dummy = wpool.tile([128, NH], BF)
            nc.vector.memset(dummy[:], 0.0)
            ident = wpool.tile([128, 128], BF)
            nc.vector.memset(ident[:], 0.0)

            # --- head DMAs, first-use order, two parallel bulk queues:
            # sync queue:   encTh(b0,h0) per-kt head chunks, then the rest of
            #               encTh; later ctx out.
            # gpsimd queue: htE (tiny), w2tk per-kt, enc b0/b1; attT out.
            # encT[b][half] -> list of (tile, kt_base, n_kt)
            encT = [[None, None] for _ in range(BPC)]

            def load_encT(b, half, kt_chunks):
                chunks = []
                kt0 = 0
                for n in kt_chunks:
                    et = etpool.tile(
                        [128, n * NH], BF, tag="encT", name=f"encT_{b}_{half}_{kt0}"
                    )
                    nc.sync.dma_start(
                        out=et[:],
                        in_=encTh_d[b, half, :, kt0 * NH : (kt0 + n) * NH],
                    )
                    chunks.append((et, kt0, n))
                    kt0 += n
                assert kt0 <= KT
                encT[b][half] = chunks

            def encT_slice(b, half, kt):
                for et, kt0, n in encT[b][half]:
                    if kt0 <= kt < kt0 + n:
                        off = (kt - kt0) * NH
                        return et[:, off : off + NH]
                raise AssertionError

            # critical head: encTh(b0,h0) per-kt chunks on the (fast) sync
            # queue; w2tk per-kt on gpsimd; prefetch follows on both. Starting
            # pass 1 earlier than the input streams can sustain only creates
            # PE gaps (and HAM re-throttle) -- this split paces perfectly.
            load_encT(0, 0, [1] * KT)
            w2tk = [None] * KT
            htE = wpool.tile([128, MT * BPC], F32)
            for kt in range(KT):
                wt = w2pool.tile([128, H], BF, tag="w2tk", name=f"w2tk_{kt}")
                nc.gpsimd.dma_start(
                    out=wt[:], in_=w2tk_d[:, kt * H : (kt + 1) * H]
                )
                w2tk[kt] = wt
                if kt == 1:
                    # htE is tiny and not needed until the first tanh (~27us)
                    nc.gpsimd.dma_start(out=htE[:], in_=htE_d[:])
            load_encT(0, 1, [2, 2, 2, 2])
            load_encT(1, 0, [4, 4])
            load_encT(1, 1, [4, 4])
            # enc (mm2 rhs, needed from ~68us) rides the TAIL of the fast
            # sync queue: in-order queue execution rate-limits it so its 4MiB
            # cannot steal HBM bandwidth from the deadline-critical encT/w2tk
            # streams above
            enc = [[None] * (KT // 4) for _ in range(BPC)]
            for b in range(BPC):
                for kq in range(KT // 4):
                    e = enpool.tile(
                        [128, 4 * H], BF, tag="enc", name=f"enc_{b}_{kq}"
                    )
                    nc.sync.dma_start(
                        out=e[:], in_=enc_d[b, :, kq * 4 * H : (kq + 1) * 4 * H]
                    )
                    enc[b][kq] = e

            def w2slice(kt, mt):
                return w2tk[kt][:, mt * 128 : (mt + 1) * 128]

            # --- PE warmup: keep HAM warm while the first chunks stream in.
            # A few big matmuls warm the clock, then small 128-col ones give
            # fine-grained bridging so the handoff to real work is tight.
            warmp = ps.tile([128, NH], F32, tag="ps")
            for i in range(WARM_BIG):
                nc.tensor.matmul(
                    warmp[:], ident[:], dummy[:], start=(i == 0), stop=(i == WARM_BIG - 1)
                )
            for i in range(WARM_SMALL):
                nc.tensor.matmul(
                    warmp[:, :128], ident[:], dummy[:, :128],
                    start=(i == 0), stop=(i == WARM_SMALL - 1),
                )

            def mm1(b):
                # energyT[h,s] block-row mt: sum_kt w2tk[kt]^T @ encT[kt]
                pss = [None] * MT
                eTs = [None] * MT
                for half in range(2):
                    cs0 = half * NH
                    for mt in range(MT):
                        if half == 0:
                            eTs[mt] = epool.tile(
                                [128, S], F32, tag="eT", name=f"eT_{b}_{mt}"
                            )
                    for kt in range(KT):
                        et = encT_slice(b, half, kt)
                        for mt in range(MT):
                            if kt == 0:
                                pss[mt] = ps.tile(
                                    [128, NH], F32, tag="ps", name=f"ps_{b}_{half}_{mt}"
                                )
                            nc.tensor.matmul(
                                pss[mt][:],
                                w2slice(kt, mt),
                                et,
                                start=(kt == 0),
                                stop=(kt == KT - 1),
                            )
                            if kt == KT - 1:
                                # evacuate: eT = tanh(psum + htE[:,mt,b])
                                nc.scalar.activation(
                                    eTs[mt][:, cs0 : cs0 + NH],
                                    pss[mt][:],
                                    AF.Tanh,
                                    bias=htE[:, mt * BPC + b : mt * BPC + b + 1],
                                )
                return eTs

            def softmax(b, eTs):
                sums = spool.tile([128, MT], F32, tag="sums")
                rec = spool.tile([128, MT], F32, tag="rec")
                ats = []
                for mt in range(MT):
                    ex = xpool.tile([128, S], BF, tag="ex")
                    nc.scalar.activation(
                        ex[:], eTs[mt][:], AF.Exp, accum_out=sums[:, mt : mt + 1]
                    )
                    nc.vector.reciprocal(rec[:, mt : mt + 1], sums[:, mt : mt + 1])
                    at = apool.tile([128, S], BF, tag="at")
                    nc.vector.tensor_scalar_mul(at[:], ex[:], rec[:, mt : mt + 1])
                    nc.gpsimd.dma_start(
                        out=attT_d[b, mt * 128 : (mt + 1) * 128, :], in_=at[:]
                    )
                    ats.append(at)
                return ats

            def mm2(b, ats):
                for mt2 in range(MT):
                    p0 = ps.tile([128, NH], F32, tag="ps", name=f"p0_{b}_{mt2}")
                    p1 = ps.tile([128, NH], F32, tag="ps", name=f"p1_{b}_{mt2}")
                    last = b == 1 and mt2 == MT - 1
                    if last:
                        # sequence p0's 8 matmuls before p1's: p0 evacuates
                        # and DMAs while p1 still multiplies
                        for kt in range(KT):
                            nc.tensor.matmul(
                                p0[:],
                                ats[kt][:, mt2 * 128 : (mt2 + 1) * 128],
                                enc[b][kt // 4][:, (kt % 4) * H : (kt % 4) * H + NH],
                                start=(kt == 0), stop=(kt == KT - 1),
                            )
                        s0 = cstg.tile([128, NH], BF, tag="cstg")
                        nc.scalar.copy(out=s0[:], in_=p0[:])
                        nc.sync.dma_start(out=ctxt_d[0], in_=s0[:])
                        for kt in range(KT):
                            nc.tensor.matmul(
                                p1[:],
                                ats[kt][:, mt2 * 128 : (mt2 + 1) * 128],
                                enc[b][kt // 4][:, (kt % 4) * H + NH : (kt % 4) * H + H],
                                start=(kt == 0), stop=(kt == KT - 1),
                            )
                        s1 = cstg.tile([128, NH], BF, tag="cstg")
                        nc.vector.tensor_copy(s1[:], p1[:])
                        nc.sync.dma_start(out=ctxt_d[1], in_=s1[:])
                        continue
                    for kt in range(KT):
                        lhs = ats[kt][:, mt2 * 128 : (mt2 + 1) * 128]
                        en = enc[b][kt // 4]
                        eo = (kt % 4) * H
                        nc.tensor.matmul(
                            p0[:], lhs, en[:, eo : eo + NH],
                            start=(kt == 0), stop=(kt == KT - 1),
                        )
                        nc.tensor.matmul(
                            p1[:], lhs, en[:, eo + NH : eo + H],
                            start=(kt == 0), stop=(kt == KT - 1),
                        )
                    # separate half tiles so each DMA depends only on its
                    # own copy
                    s0 = cstg.tile([128, NH], BF, tag="cstg")
                    s1 = cstg.tile([128, NH], BF, tag="cstg")
                    nc.scalar.copy(out=s0[:], in_=p0[:])
                    nc.sync.dma_start(
                        out=ctx_d[b, mt2 * 128 : (mt2 + 1) * 128, :NH],
                        in_=s0[:],
                    )
                    nc.vector.tensor_copy(s1[:], p1[:])
                    nc.gpsimd.dma_start(
                        out=ctx_d[b, mt2 * 128 : (mt2 + 1) * 128, NH:],
                        in_=s1[:],
                    )

            eT0 = mm1(0)
            a0 = softmax(0, eT0)
            eT1 = mm1(1)
            mm2(0, a0)
            a1 = softmax(1, eT1)
            mm2(1, a1)

    _split_sync_waits(nc, 1)
    return nc


_NC_CACHE = {}


def _get_nc():
    if "nc" not in _NC_CACHE:
        _NC_CACHE["nc"] = build()
    return _NC_CACHE["nc"]


def _pack(m):
    # [1024, D] -> [128, 8*D] with 128-row tile kt at columns [kt*D,(kt+1)*D)
    d = m.shape[1]
    return np.ascontiguousarray(
        m.reshape(KT, 128, d).transpose(1, 0, 2).reshape(128, KT * d)
    )


def _make_in_maps(ht, enc, W_attn, b_attn):
    import ml_dtypes

    bf = ml_dtypes.bfloat16
    ht = np.asarray(ht, np.float32)
    enc = np.asarray(enc, np.float32)
    W = np.asarray(W_attn, np.float32)
    ba = np.asarray(b_attn, np.float32)

    # w2tk[p, kt*H + mt*128 + j] = W2T[kt*128+p, mt*128+j] (kt-major packing)
    w2tk_p = _pack(W[:, H:].T.copy()).astype(bf)
    # htE_full[b, h] = ht @ W1.T + b_attn  (computed on host, tiny)
    htE_full = (ht @ W[:, :H].T + ba).astype(np.float32)  # [B, H]

    in_maps = []
    for c in range(NCORES):
        bs = slice(BPC * c, BPC * (c + 1))
        enc_c = enc[bs]
        enc_p = np.stack([_pack(enc_c[i]) for i in range(BPC)]).astype(bf)
        # encTh[b, half, p, kt*NH + j] = enc_c[b][half*NH + j, kt*128 + p]
        encTh_p = np.ascontiguousarray(
            enc_c.reshape(BPC, 2, NH, KT, 128).transpose(0, 1, 4, 3, 2)
        ).reshape(BPC, 2, 128, KT * NH).astype(bf)
        # htE_col[p, mt*BPC + i] = htE_full[bs][i, mt*128 + p]
        htE_c = np.ascontiguousarray(
            htE_full[bs].reshape(BPC, MT, 128).transpose(2, 1, 0).reshape(128, MT * BPC)
        )
        in_maps.append(
            {"enc": enc_p, "encTh": encTh_p, "w2tk": w2tk_p, "htE": htE_c}
        )
    return in_maps


def _run(in_maps, trace=False):
    res = run_bass_kernel_spmd(
        _get_nc(), in_maps, core_ids=list(range(NCORES)), trace=trace
    )
    ctx_parts = []
    for r in res.results:
        c = r["ctx"].copy()
        tail = r["ctxt"]  # [2, 128, NH]: last row-block of b1, col-halves
        c[BPC - 1, (MT - 1) * 128 :, :NH] = tail[0]
        c[BPC - 1, (MT - 1) * 128 :, NH:] = tail[1]
        ctx_parts.append(c.astype(np.float32))
    ctx = np.concatenate(ctx_parts, axis=0)
    att = np.concatenate(
        [r["attT"].transpose(0, 2, 1).astype(np.float32) for r in res.results],
        axis=0,
    )
    return (ctx, att), res


def kernel(ht, encoder_out, W_attn, b_attn, W_v=None, **_unused):
    out, _ = _run(_make_in_maps(ht, encoder_out, W_attn, b_attn), trace=False)
    return out


def kernel_traced(ht, encoder_out, W_attn, b_attn, W_v=None, **_unused):
    """Like kernel() but also returns the BassKernelResults with profile."""
    out, res = _run(_make_in_maps(ht, encoder_out, W_attn, b_attn), trace=True)
    return out, res


# revision 56
# speedup vs baseline: 1.1312x; 1.0137x over previous
"""Trainium2 Bass kernel for nn_AttentionNetwork (B=16, S=H=1024).

reference:
    energy  = tanh(concat([ht bcast, enc], -1) @ W_attn.T + b_attn)   [B,S,H]
    att     = softmax(energy, axis=1)  (over the seq axis)
    context = einsum('bsk,bkh->bsh', att, enc)
    returns (context, att)   (the W_v projection output is dead code)

Strategy (final):
  - Data-parallel over batch: 2 batches per NeuronCore x 8 cores (SPMD).
  - htE = ht @ W1.T + b_attn computed on HOST (tiny GEMM), shipped as an
    8KB f32 tensor; device only does the two big GEMMs per batch.
  - mm1 computes energy TRANSPOSED (energyT[h,s]) so softmax over s is a
    free-dim reduction. kt-OUTER loop order over all 8 row-blocks (8 PSUM
    banks): each arriving chunk unlocks a whole row of work, so pass 1 is
    perfectly DMA-paced with no PE gaps. encT is packed HALF-MAJOR on the
    host and the critical (b0, half0) stream uses PER-KT 128KB chunks.
  - Input streams ride two DMA queues in parallel (sync: encT; gpsimd:
    htE/w2tk/enc-prefetch); the sync queue is empirically much faster, so
    it carries the tightest deadlines. Starting the PE earlier than the
    streams can sustain only creates gaps + HAM re-throttle; instead PE
    warm-up dummies (8 big + 18 small) bridge exactly until first data.
  - softmax: tanh(+bias) PSUM->SBUF f32 on ACT, exp with accum_out sums,
    reciprocal + at = ex*rec (bf16) on DVE (GpSimd tensor ops are slow
    and also wedge DVE's fast mode -- keep GpSimd to DMA triggers only).
  - att is written to DRAM TRANSPOSED (attT[h,s], bf16) straight from the
    at tiles -- no PE transposes; the host transposes + upcasts.
  - mm2: ctx = matmul(lhsT=at, rhs=enc) natural [s,h]; PSUM evacuated as
    bf16 via scalar/vector copy halves; DMA out bf16 (halving write
    traffic), host upcasts. The final row-block runs its two column
    halves sequentially and drains into contiguous side tensors so the
    last DMA is a short 1D burst overlapped with the last matmuls.
  - Engine queues: scalar = activations/copies ONLY (DMA triggers would
    head-of-line block the tanh chain); sync = encT in + ctx out;
    gpsimd = htE/w2tk/enc in + attT out; vector = recip/at-mult/ctx-copy.
  - PE program order: warm dummies, mm1(b0), mm1(b1), mm2(b0), mm2(b1)
    with no PE gaps; everything else rides in the matmul shadow.
  (fp8/DoubleRow was evaluated and rejected: only e4m3/e5m2 double-pump,
  and their quantization noise puts ctx relL2 at ~4e-2 vs the 2e-2 gate.)
"""

import sys
import numpy as np

sys.path.insert(0, "/opt/trn_rl_repo")

import concourse.bass as bass
import concourse.mybir as mybir
import concourse.tile as tile
from concourse.bass_utils import run_bass_kernel_spmd

F32 = mybir.dt.float32
BF = mybir.dt.bfloat16
AF = mybir.ActivationFunctionType

B, S, H = 16, 1024, 1024
NCORES = 8
BPC = B // NCORES  # batches per core
KT = 8             # 128-row contraction tiles
MT = 8             # output partition tiles
NH = 512           # matmul free-dim chunk (one PSUM bank fp32)
WARM_BIG = 8       # [128,512] dummy matmuls warming the PE clock
WARM_SMALL = 18    # [128,128] dummy matmuls bridging until first data


def _split_sync_waits(nc, maxw=1):
    """This walrus rejects instructions with more than one sync wait.
    Move excess on_wait entries onto InstNoOp on the same engine queue
    (executed in order ahead of the real instruction)."""
    ctr = 0
    for fn in nc.m.functions:
        for blk in fn.blocks:
            new = []
            for inst in blk.instructions:
                si = inst.sync_info
                if si is not None and si.on_wait and len(si.on_wait) > maxw:
                    waits = list(si.on_wait)
                    extra, keep = waits[:-maxw], waits[-maxw:]
                    for i in range(0, len(extra), maxw):
                        ctr += 1
                        nop = mybir.InstNoOp(
                            name=f"I-ws-{ctr}",
                            engine=inst.engine,
                            sync_info=mybir.SyncInfo(
                                on_wait=extra[i : i + maxw], on_update=[]
                            ),
                        )
                        nc.register_instruction(nop)
                        new.append(nop)
                    inst.sync_info = mybir.SyncInfo(
                        on_wait=keep, on_update=list(si.on_update)
                    )
                new.append(inst)
            blk.instructions[:] = new
    return ctr


def build():
    nc = bass.Bass()
    # encTh[b, half, p, kt*NH + j] = enc[b][half*NH + j, kt*128 + p]
    encTh_d = nc.declare_dram_parameter(
        "encTh", [BPC, 2, 128, KT * NH], BF, isOutput=False
    )
    enc_d = nc.declare_dram_parameter("enc", [BPC, 128, KT * H], BF, isOutput=False)
    w2tk_d = nc.declare_dram_parameter("w2tk", [128, KT * H], BF, isOutput=False)
    htE_d = nc.declare_dram_parameter("htE", [128, MT * BPC], F32, isOutput=False)
    ctx_d = nc.declare_dram_parameter("ctx", [BPC, S, H], BF, isOutput=True)
    # the final row-block's two column-halves as contiguous tensors: the very
    # last DMA is a fast 1D 128KB burst instead of a strided 2D write
    ctxt_d = nc.declare_dram_parameter("ctxt", [2, 128, NH], BF, isOutput=True)
    attT_d = nc.declare_dram_parameter("attT", [BPC, H, S], BF, isOutput=True)

    with tile.TileContext(nc) as tc:
        with (
            tc.tile_pool(name="wpool", bufs=1) as wpool,
            tc.tile_pool(name="w2pool", bufs=KT) as w2pool,           # w2tk per-kt
            tc.tile_pool(name="etpool", bufs=16) as etpool,           # encT chunks
            tc.tile_pool(name="enpool", bufs=2 * KT // 4) as enpool,  # enc 4-kt
            tc.tile_pool(name="epool", bufs=MT) as epool,             # eT f32
            tc.tile_pool(name="xpool", bufs=3) as xpool,              # ex bf16
            tc.tile_pool(name="apool", bufs=2 * KT) as apool,         # at bf16
            tc.tile_pool(name="spool", bufs=4) as spool,              # sums/rec
            tc.tile_pool(name="cstg", bufs=4) as cstg,                # ctx staging
            tc.tile_pool(name="ps", bufs=8, space="PSUM") as ps,      # all 8 banks
        ):
            # --- ACT table warm first: nothing sits in front of it on the
            # scalar queue, so the 1.3us spline-table load happens at ~4us.
            warma = wpool.tile([128, 1], F32)
            nc.vector.memset(warma[:], 0.5)
            nc.scalar.activation(warma[:], warma[:], AF.Exp)

            # --- warmup prerequisites before any DMA trigger so the PE can
            # start its HAM-warming dummies as early as possible (both on the
            # vector queue, which is otherwise idle at the head).
            dummy = wpool.tile([128, NH], BF)
            nc.vector.memset(dummy[:], 0.0)
            ident = wpool.tile([128, 128], BF)
            nc.vector.memset(ident[:], 0.0)

            # --- head DMAs, first-use order, two parallel bulk queues:
            # sync queue:   encTh(b0,h0) per-kt head chunks, then the rest of
            #               encTh; later ctx out.
            # gpsimd queue: htE (tiny), w2tk per-kt, enc b0/b1; attT out.
            # encT[b][half] -> list of (tile, kt_base, n_kt)
            encT = [[None, None] for _ in range(BPC)]

            def load_encT(b, half, kt_chunks):
                chunks = []
                kt0 = 0
                for n in kt_chunks:
                    et = etpool.tile(
                        [128, n * NH], BF, tag="encT", name=f"encT_{b}_{half}_{kt0}"
                    )
                    nc.sync.dma_start(
                        out=et[:],
                        in_=encTh_d[b, half, :, kt0 * NH : (kt0 + n) * NH],
                    )
                    chunks.append((et, kt0, n))
                    kt0 += n
                assert kt0 <= KT
                encT[b][half] = chunks

            def encT_slice(b, half, kt):
                for et, kt0, n in encT[b][half]:
                    if kt0 <= kt < kt0 + n:
                        off = (kt - kt0) * NH
                        return et[:, off : off + NH]
                raise AssertionError

            # critical head: encTh(b0,h0) per-kt chunks on the (fast) sync
            # queue; w2tk per-kt on gpsimd; prefetch follows on both. Starting
            # pass 1 earlier than the input streams can sustain only creates
            # PE gaps (and HAM re-throttle) -- this split paces perfectly.
            load_encT(0, 0, [1] * KT)
            w2tk = [None] * KT
            htE = wpool.tile([128, MT * BPC], F32)
            for kt in range(KT):
                wt = w2pool.tile([128, H], BF, tag="w2tk", name=f"w2tk_{kt}")
                nc.gpsimd.dma_start(
                    out=wt[:], in_=w2tk_d[:, kt * H : (kt + 1) * H]
                )
                w2tk[kt] = wt
                if kt == 1:
                    # htE is tiny and not needed until the first tanh (~27us)
                    nc.gpsimd.dma_start(out=htE[:], in_=htE_d[:])
            load_encT(0, 1, [2, 2, 2, 2])
            load_encT(1, 0, [4, 4])
            load_encT(1, 1, [4, 4])
            # enc (mm2 rhs, needed from ~68us) rides the TAIL of the fast
            # sync queue: in-order queue execution rate-limits it so its 4MiB
            # cannot steal HBM bandwidth from the deadline-critical encT/w2tk
            # streams above
            enc = [[None] * (KT // 4) for _ in range(BPC)]
            for b in range(BPC):
                for kq in range(KT // 4):
                    e = enpool.tile(
                        [128, 4 * H], BF, tag="enc", name=f"enc_{b}_{kq}"
                    )
                    nc.sync.dma_start(
                        out=e[:], in_=enc_d[b, :, kq * 4 * H : (kq + 1) * 4 * H]
                    )
                    enc[b][kq] = e

            def w2slice(kt, mt):
                return w2tk[kt][:, mt * 128 : (mt + 1) * 128]

            # --- PE warmup: keep HAM warm while the first chunks stream in.
            # A few big matmuls warm the clock, then small 128-col ones give
            # fine-grained bridging so the handoff to real work is tight.
            warmp = ps.tile([128, NH], F32, tag="ps")
            for i in range(WARM_BIG):
                nc.tensor.matmul(
                    warmp[:], ident[:], dummy[:], start=(i == 0), stop=(i == WARM_BIG - 1)
                )
            for i in range(WARM_SMALL):
                nc.tensor.matmul(
                    warmp[:, :128], ident[:], dummy[:, :128],
                    start=(i == 0), stop=(i == WARM_SMALL - 1),
                )

            def mm1(b):
                # energyT[h,s] block-row mt: sum_kt w2tk[kt]^T @ encT[kt]
                pss = [None] * MT
                eTs = [None] * MT
                for half in range(2):
                    cs0 = half * NH
                    for mt in range(MT):
                        if half == 0:
                            eTs[mt] = epool.tile(
                                [128, S], F32, tag="eT", name=f"eT_{b}_{mt}"
                            )
                    for kt in range(KT):
                        et = encT_slice(b, half, kt)
                        for mt in range(MT):
                            if kt == 0:
                                pss[mt] = ps.tile(
                                    [128, NH], F32, tag="ps", name=f"ps_{b}_{half}_{mt}"
                                )
                            nc.tensor.matmul(
                                pss[mt][:],
                                w2slice(kt, mt),
                                et,
                                start=(kt == 0),
                                stop=(kt == KT - 1),
                            )
                            if kt == KT - 1:
                                # evacuate: eT = tanh(psum + htE[:,mt,b])
                                nc.scalar.activation(
                                    eTs[mt][:, cs0 : cs0 + NH],
                                    pss[mt][:],
                                    AF.Tanh,
                                    bias=htE[:, mt * BPC + b : mt * BPC + b + 1],
                                )
                return eTs

            def softmax(b, eTs):
                sums = spool.tile([128, MT], F32, tag="sums")
                rec = spool.tile([128, MT], F32, tag="rec")
                ats = []
                for mt in range(MT):
                    ex = xpool.tile([128, S], BF, tag="ex")
                    nc.scalar.activation(
                        ex[:], eTs[mt][:], AF.Exp, accum_out=sums[:, mt : mt + 1]
                    )
                    nc.vector.reciprocal(rec[:, mt : mt + 1], sums[:, mt : mt + 1])
                    at = apool.tile([128, S], BF, tag="at")
                    nc.vector.tensor_scalar_mul(at[:], ex[:], rec[:, mt : mt + 1])
                    nc.gpsimd.dma_start(
                        out=attT_d[b, mt * 128 : (mt + 1) * 128, :], in_=at[:]
                    )
                    ats.append(at)
                return ats

            def mm2(b, ats):
                for mt2 in range(MT):
                    p0 = ps.tile([128, NH], F32, tag="ps", name=f"p0_{b}_{mt2}")
                    p1 = ps.tile([128, NH], F32, tag="ps", name=f"p1_{b}_{mt2}")
                    last = b == 1 and mt2 == MT - 1
                    if last:
                        # sequence p0's 8 matmuls before p1's: p0 evacuates
                        # and DMAs while p1 still multiplies
                        for kt in range(KT):
                            nc.tensor.matmul(
                                p0[:],
                                ats[kt][:, mt2 * 128 : (mt2 + 1) * 128],
                                enc[b][kt // 4][:, (kt % 4) * H : (kt % 4) * H + NH],
                                start=(kt == 0), stop=(kt == KT - 1),
                            )
                        s0 = cstg.tile([128, NH], BF, tag="cstg")
                        nc.scalar.copy(out=s0[:], in_=p0[:])
                        nc.sync.dma_start(out=ctxt_d[0], in_=s0[:])
                        for kt in range(KT):
                            nc.tensor.matmul(
                                p1[:],
                                ats[kt][:, mt2 * 128 : (mt2 + 1) * 128],
                                enc[b][kt // 4][:, (kt % 4) * H + NH : (kt % 4) * H + H],
                                start=(kt == 0), stop=(kt == KT - 1),
                            )
                        s1 = cstg.tile([128, NH], BF, tag="cstg")
                        nc.vector.tensor_copy(s1[:], p1[:])
                        nc.sync.dma_start(out=ctxt_d[1], in_=s1[:])
                        continue
                    for kt in range(KT):
                        lhs = ats[kt][:, mt2 * 128 : (mt2 + 1) * 128]
                        en = enc[b][kt // 4]
                        eo = (kt % 4) * H
                        nc.tensor.matmul(
                            p0[:], lhs, en[:, eo : eo + NH],
                            start=(kt == 0), stop=(kt == KT - 1),
                        )
                        nc.tensor.matmul(
                            p1[:], lhs, en[:, eo + NH : eo + H],
                            start=(kt == 0), stop=(kt == KT - 1),
                        )
                    # separate half tiles so each DMA depends only on its
                    # own copy
                    s0 = cstg.tile([128, NH], BF, tag="cstg")
                    s1 = cstg.tile([128, NH], BF, tag="cstg")
                    nc.scalar.copy(out=s0[:], in_=p0[:])
                    nc.sync.dma_start(
                        out=ctx_d[b, mt2 * 128 : (mt2 + 1) * 128, :NH],
                        in_=s0[:],
                    )
                    nc.vector.tensor_copy(s1[:], p1[:])
                    nc.gpsimd.dma_start(
                        out=ctx_d[b, mt2 * 128 : (mt2 + 1) * 128, NH:],
                        in_=s1[:],
                    )

            eT0 = mm1(0)
            a0 = softmax(0, eT0)
            eT1 = mm1(1)
            mm2(0, a0)
            a1 = softmax(1, eT1)
            mm2(1, a1)

    _split_sync_waits(nc, 1)
    return nc


_NC_CACHE = {}


def _get_nc():
    if "nc" not in _NC_CACHE:
        _NC_CACHE["nc"] = build()
    return _NC_CACHE["nc"]


def _pack(m):
    # [1024, D] -> [128, 8*D] with 128-row tile kt at columns [kt*D,(kt+1)*D)
    d = m.shape[1]
    return np.ascontiguousarray(
        m.reshape(KT, 128, d).transpose(1, 0, 2).reshape(128, KT * d)
    )


def _make_in_maps(ht, enc, W_attn, b_attn):
    import ml_dtypes

    bf = ml_dtypes.bfloat16
    ht = np.asarray(ht, np.float32)
    enc = np.asarray(enc, np.float32)
    W = np.asarray(W_attn, np.float32)
    ba = np.asarray(b_attn, np.float32)

    # w2tk[p, kt*H + mt*128 + j] = W2T[kt*128+p, mt*128+j] (kt-major packing)
    w2tk_p = _pack(W[:, H:].T.copy()).astype(bf)
    # htE_full[b, h] = ht @ W1.T + b_attn  (computed on host, tiny)
    htE_full = (ht @ W[:, :H].T + ba).astype(np.float32)  # [B, H]

    in_maps = []
    for c in range(NCORES):
        bs = slice(BPC * c, BPC * (c + 1))
        enc_c = enc[bs]
        enc_p = np.stack([_pack(enc_c[i]) for i in range(BPC)]).astype(bf)
        # encTh[b, half, p, kt*NH + j] = enc_c[b][half*NH + j, kt*128 + p]
        encTh_p = np.ascontiguousarray(
            enc_c.reshape(BPC, 2, NH, KT, 128).transpose(0, 1, 4, 3, 2)
        ).reshape(BPC, 2, 128, KT * NH).astype(bf)
        # htE_col[p, mt*BPC + i] = htE_full[bs][i, mt*128 + p]
        htE_c = np.ascontiguousarray(
            htE_full[bs].reshape(BPC, MT, 128).transpose(2, 1, 0).reshape(128, MT * BPC)
        )
        in_maps.append(
            {"enc": enc_p, "encTh": encTh_p, "w2tk": w2tk_p, "htE": htE_c}
        )
    return in_maps


def _run(in_maps, trace=False):
    res = run_bass_kernel_spmd(
        _get_nc(), in_maps, core_ids=list(range(NCORES)), trace=trace
    )
    ctx_parts = []
    for r in res.results:
        c = r["ctx"].copy()
        tail = r["ctxt"]  # [2, 128, NH]: last row-block of b1, col-halves
        c[BPC - 1, (MT - 1) * 128 :, :NH] = tail[0]
        c[BPC - 1, (MT - 1) * 128 :, NH:] = tail[1]
        ctx_parts.append(c.astype(np.float32))
    ctx = np.concatenate(ctx_parts, axis=0)
    att = np.concatenate(
        [r["attT"].transpose(0, 2, 1).astype(np.float32) for r in res.results],
        axis=0,
    )
    return (ctx, att), res


def kernel(ht, encoder_out, W_attn, b_attn, W_v=None, **_unused):
    out, _ = _run(_make_in_maps(ht, encoder_out, W_attn, b_attn), trace=False)
    return out


def kernel_traced(ht, encoder_out, W_attn, b_attn, W_v=None, **_unused):
    """Like kernel() but also returns the BassKernelResults with profile."""
    out, res = _run(_make_in_maps(ht, encoder_out, W_attn, b_attn), trace=True)
    return out, res
